# revision 7
# baseline (speedup 1.0000x reference)
"""GraphSAGE (2x SAGEConv mean-aggr + MLP decoder) on 8 Trainium2 NeuronCores.

v3 design (sim-trace driven; v2 was balanced PE/Pool/SP at ~80-93% each):
- dst-node sharding, 12500/core padded to 12800 (4 quarters x 3200), unified
  node numbering shared by both rounds (quarter-major AllGather layout).
- SB_NODES=128 (was 256): halves one-hot mask bytes AND the per-chunk PE
  matmul cost (rhs columns).
- Gathers merged into groups of GS=10 superbatches: 4 bank-calls per group
  (80 total vs 400) to amortize the ~1us fixed SWDGE cost per dma_gather.
  Gather slot layout is group-major/bank-major/sb-major; the fp8 one-hot
  masks are stored sb-major so each sb's mask is ONE contiguous DMA; the
  matmul loop pairs gat chunks with mask chunks via host-computed offsets.
- Mask DMAs alternate between the SP and ACT HWDGE rings (two physical
  rings; v2 serialized everything on SP at 93% busy).
- recip kept on 1 partition and partition-broadcast in the mean multiply
  (saves 25KB/partition of SBUF, enabling GS=10).
- Aggregation: per chunk one matmul lhsT=gat[slot,feat] rhs=mask[slot,128]
  accumulated into fp32 PSUM aggT[feat,dst]; mean via DVE multiply with
  broadcast 1/deg; linears in T-orientation; round 1 relu -> SBUF-resident
  h1selfT -> PE transpose -> h1tab_in; 4-chunk AllGather; decoder fused.
"""

import os

import numpy as np
import ml_dtypes

import concourse.bacc as bacc
import concourse.bass as bass
import concourse.mybir as mybir
import concourse.tile as tile
from concourse.bass_utils import run_bass_kernel_spmd
from concourse.library_config import mlp as mlp_lib

BF16 = ml_dtypes.bfloat16
FP8 = ml_dtypes.float8_e4m3fn

ABLATE = set(os.environ.get("K2_ABLATE", "").split(","))

N_CORES = 8
D = 128
P = 128
SB_NODES = 128
BANK = 25600

SHARD = 12500
QCAP = 3200           # quarter capacity (multiple of 128)
SHARD_PAD = 4 * QCAP  # 12800
N_PAD = N_CORES * SHARD_PAD  # 102400
N_SB = SHARD_PAD // SB_NODES  # 100
N_BANKS = (N_PAD + BANK - 1) // BANK  # 4
GS = 10               # superbatches per gather group
N_GROUPS = N_SB // GS  # 10


def _pad_local(r):
    """local node index [0,12500) -> quarter-padded [0,12800)."""
    q = r // 3125
    return q * QCAP + (r - q * 3125)


def _unified_row(v):
    """global node id -> row in the unified padded table.

    Quarter-major: row = q*8*QCAP + core*QCAP + r_within_quarter, matching
    the layout the 4-chunk AllGather produces (chunk q = concat over cores
    of their quarter q), so AG chunk q fills exactly gather bank q."""
    c = v // SHARD
    r = v - c * SHARD
    q = r // 3125
    rq = r - q * 3125
    return q * (N_CORES * QCAP) + c * QCAP + rq


def _slot_meta(src_row, dst_pad, core_of_edge):
    """Group each core's edges by (sb, bank, dst); pad per-(sb,bank)
    segments to a common (max-over-cores, 128-aligned) budget.

    Gather slot order: group-major, bank-major within group, sb-major
    within bank (so each (group, bank) is one contiguous dma_gather).
    Mask slot order: sb-major (so each sb's mask is one contiguous DMA).
    Returns per-core idx (int16 wrapped) + per-core fp8 one-hot masks
    [128, n_chunks, SB_NODES] + shared budgets/offsets."""
    sb = dst_pad // SB_NODES
    bank = src_row // BANK

    counts = np.zeros((N_CORES, N_SB, N_BANKS), dtype=np.int64)
    np.add.at(counts, (core_of_edge, sb, bank), 1)
    budgets = counts.max(axis=0)
    budgets = ((budgets + 127) // 128) * 128

    seg_off = np.zeros((N_SB, N_BANKS), dtype=np.int64)
    group_base = np.zeros(N_GROUPS + 1, dtype=np.int64)
    pos = 0
    for g in range(N_GROUPS):
        group_base[g] = pos
        for b in range(N_BANKS):
            for s in range(g * GS, (g + 1) * GS):
                seg_off[s, b] = pos
                pos += budgets[s, b]
    group_base[N_GROUPS] = pos
    total_slots = int(pos)
    n_chunks = total_slots // 128

    mask_seg_off = np.zeros((N_SB, N_BANKS), dtype=np.int64)
    mpos = 0
    for s in range(N_SB):
        for b in range(N_BANKS):
            mask_seg_off[s, b] = mpos
            mpos += budgets[s, b]
    assert mpos == total_slots

    idx_cores, mask_cores = [], []
    for c in range(N_CORES):
        m = core_of_edge == c
        s_c, dp_c, sb_c, bk_c = (src_row[m], dst_pad[m], sb[m], bank[m])
        order = np.lexsort((dp_c, bk_c, sb_c))
        s_c, dp_c, sb_c, bk_c = (a[order] for a in (s_c, dp_c, sb_c, bk_c))

        idx_full = np.zeros(total_slots, dtype=np.int16)
        dstw_full = np.full(total_slots, -1, dtype=np.int64)
        cnt_c = np.zeros((N_SB, N_BANKS), dtype=np.int64)
        np.add.at(cnt_c, (sb_c, bk_c), 1)
        # edges are sorted (sb, bank): per-segment start in that order
        seg_start = np.zeros((N_SB, N_BANKS), dtype=np.int64)
        seg_start.reshape(-1)[1:] = np.cumsum(cnt_c.reshape(-1))[:-1]
        pos_in_seg = np.arange(len(s_c)) - seg_start[sb_c, bk_c]
        gslot = seg_off[sb_c, bk_c] + pos_in_seg
        mslot = mask_seg_off[sb_c, bk_c] + pos_in_seg
        idx_full[gslot] = (s_c - bk_c * BANK).astype(np.int16)
        dstw_full[mslot] = dp_c - sb_c * SB_NODES

        # idx wrap: slot i -> [i%16, i//16], replicated to 128 partitions
        w = idx_full.reshape(total_slots // 16, 16).T
        idx_cores.append(np.tile(w, (8, 1)).copy())

        # fp8 one-hot mask (sb-major): slot i -> partition i%128, chunk i//128
        dw = dstw_full.reshape(n_chunks, 128).T          # [128, chunks]
        mask = np.zeros((P, n_chunks, SB_NODES), dtype=FP8)
        valid = dw >= 0
        np.put_along_axis(mask, dw.clip(0)[:, :, None],
                          valid[:, :, None].astype(FP8), axis=2)
        mask_cores.append(mask)

    return {
        "budgets": budgets, "seg_off": seg_off, "mask_seg_off": mask_seg_off,
        "group_base": group_base, "total_slots": total_slots,
        "n_chunks": n_chunks, "idx": idx_cores, "mask": mask_cores,
    }


def prep(inputs):
    x = np.asarray(inputs["x"], dtype=np.float32)
    ei = np.asarray(inputs["edge_index"])
    n = x.shape[0]
    assert n == N_CORES * SHARD

    src = ei[0].astype(np.int64)
    dst = ei[1].astype(np.int64)
    src_row = _unified_row(src)
    core_of_edge = dst // SHARD
    dst_local = dst - core_of_edge * SHARD
    dst_pad = _pad_local(dst_local)

    sm = _slot_meta(src_row, dst_pad, core_of_edge)

    deg = np.bincount(dst, minlength=n).astype(np.float32)
    recip = (1.0 / np.maximum(deg, 1.0)).astype(np.float32)
    recip_pad = np.zeros((N_CORES, 1, SHARD_PAD), dtype=np.float32)
    x_selfT = np.zeros((N_CORES, D, SHARD_PAD), dtype=np.float32)
    for c in range(N_CORES):
        rl = _pad_local(np.arange(SHARD))
        recip_pad[c, 0, rl] = recip[c * SHARD:(c + 1) * SHARD]
        x_selfT[c, :, rl] = x[c * SHARD:(c + 1) * SHARD, :]

    x_pad = np.zeros((N_PAD, D), dtype=np.float32)
    x_pad[_unified_row(np.arange(n))] = x

    meta = {
        "sm": sm,
        "x_pad": x_pad.astype(BF16),
        "x_selfT": x_selfT.astype(BF16),
        "recip": recip_pad.astype(BF16),
        "W1_lT": np.asarray(inputs["W1_l"], np.float32).T.astype(BF16).copy(),
        "W1_rT": np.asarray(inputs["W1_r"], np.float32).T.astype(BF16).copy(),
        "W2_lT": np.asarray(inputs["W2_l"], np.float32).T.astype(BF16).copy(),
        "W2_rT": np.asarray(inputs["W2_r"], np.float32).T.astype(BF16).copy(),
        "W3T": np.asarray(inputs["W3"], np.float32).T.astype(BF16).copy(),
        "W4c": np.asarray(inputs["W4"], np.float32).reshape(2, 128).T
            .astype(BF16).copy(),
        "b1c": np.asarray(inputs["b1"], np.float32).reshape(-1, 1)
            .astype(BF16).copy(),
        # conv2 has no relu, so b2 folds exactly into the decoder bias:
        # relu(W3 @ (h2 + b2) + b3) = relu(W3 @ h2 + (b3 + W3 @ b2))
        "b3c": (np.asarray(inputs["b3"], np.float32)
                + np.asarray(inputs["W3"], np.float32)
                @ np.asarray(inputs["b2"], np.float32))
            .reshape(2, 128).T.astype(BF16).copy(),
        "b4": float(np.asarray(inputs["b4"]).reshape(-1)[0]),
        "ident": np.eye(P, dtype=BF16),
    }
    meta["recip_bc"] = np.broadcast_to(
        meta["recip"], (N_CORES, P, SHARD_PAD)).copy()
    return meta


WNAMES = ["W1_lT", "W1_rT", "W2_lT", "W2_rT", "W3T", "W4c",
          "b1c", "b3c", "ident"]


def build(meta):
    sm = meta["sm"]
    nc = bacc.Bacc("TRN2", target_bir_lowering=False, debug=False,
                   num_devices=N_CORES, num_swdge_queues=4)
    f32, bf16, fp8 = mybir.dt.float32, mybir.dt.bfloat16, mybir.dt.float8e4
    i16 = mybir.dt.int16

    x_tab = nc.dram_tensor("x_tab", [N_PAD, D], bf16, kind="ExternalInput")
    x_selfT_d = nc.dram_tensor("x_selfT", [D, SHARD_PAD], bf16,
                               kind="ExternalInput")
    recip_d = nc.dram_tensor("recip_bc", [P, SHARD_PAD], bf16,
                              kind="ExternalInput")
    idx_d = nc.dram_tensor("idx", list(sm["idx"][0].shape), i16,
                           kind="ExternalInput")
    mask_d = nc.dram_tensor("mask", [P, sm["n_chunks"], SB_NODES], fp8,
                            kind="ExternalInput")
    wt = {}
    for name in WNAMES:
        arr = meta[name]
        wt[name] = nc.dram_tensor(name, list(arr.shape), bf16,
                                  kind="ExternalInput")

    h1tab_in = nc.dram_tensor("h1tab_in", [SHARD_PAD, D], bf16)
    h1tab = nc.dram_tensor("h1tab", [N_PAD, D], bf16,
                           addr_space="Shared")
    out_shard = nc.dram_tensor("out_shard", [N_SB, SB_NODES], f32,
                               kind="ExternalOutput")

    budgets, seg_off = sm["budgets"], sm["seg_off"]
    mask_seg_off, group_base = sm["mask_seg_off"], sm["group_base"]

    with tile.TileContext(nc) as tc:
        with (
            tc.tile_pool(name="wp", bufs=1) as wp,
            tc.tile_pool(name="gp", bufs=2) as gp,
            tc.tile_pool(name="sp", bufs=4) as sp,
            tc.tile_pool(name="vp", bufs=3) as vp,
            tc.tile_pool(name="psA", bufs=2, space="PSUM") as psA,
            tc.tile_pool(name="psB", bufs=1, space="PSUM") as psB,
            tc.tile_pool(name="psM", bufs=2, space="PSUM") as psM,
            tc.tile_pool(name="psD", bufs=1, space="PSUM") as psD,
        ):
            with tc.tile_critical():
                nc.gpsimd.load_library(mlp_lib)

            consts = {}
            for name in WNAMES:
                t = wp.tile(list(meta[name].shape), bf16, tag=name)
                nc.sync.dma_start(t[:], wt[name][:])
                consts[name] = t
            recip_t = wp.tile([P, SHARD_PAD], bf16, tag="recip")
            nc.sync.dma_start(recip_t[:], recip_d[:])
            x_selfT_t = wp.tile([D, SHARD_PAD], bf16, tag="xselfT")
            nc.sync.dma_start(x_selfT_t[:], x_selfT_d[:])
            h1selfT_t = wp.tile([D, SHARD_PAD], bf16, tag="h1selfT")

            def emit_round(rnd, table, wl, wr, brow):
                selfT = x_selfT_t if rnd == 0 else h1selfT_t
                for g in range(N_GROUPS):
                    gbase = int(group_base[g])
                    gslots = int(group_base[g + 1]) - gbase

                    idx_t = gp.tile([P, gslots // 16], i16, tag="idx")
                    nc.sync.dma_start(
                        idx_t[:],
                        idx_d[:, gbase // 16: (gbase + gslots) // 16])
                    gat = gp.tile([P, gslots // 128, D], bf16, tag="gat")
                    for b in range(N_BANKS):
                        nb = int(budgets[g * GS:(g + 1) * GS, b].sum())
                        if nb == 0 or "gather" in ABLATE:
                            continue
                        off = int(seg_off[g * GS, b]) - gbase
                        lo = b * BANK
                        hi = min(N_PAD, (b + 1) * BANK)
                        nc.gpsimd.dma_gather(
                            gat[:, off // 128: (off + nb) // 128, :],
                            table[lo:hi, :],
                            idx_t[:, off // 16: (off + nb) // 16],
                            num_idxs=nb, num_idxs_reg=nb, elem_size=D,
                            single_packet=False, queue_num=b,
                        )

                    for s in range(g * GS, (g + 1) * GS):
                        c_sb = int(budgets[s].sum()) // 128
                        mc0 = int(mask_seg_off[s, 0]) // 128
                        mask_t = sp.tile([P, c_sb, SB_NODES], fp8, tag="mask")
                        ring = nc.sync if s % 2 == 0 else nc.scalar
                        ring.dma_start(mask_t[:], mask_d[:, mc0:mc0 + c_sb, :])

                        pa = psA.tile([P, SB_NODES], f32, tag="pa")
                        jj = 0
                        lim = 1 if "agg" in ABLATE else c_sb
                        for b in range(N_BANKS):
                            nb = int(budgets[s, b])
                            lc0 = (int(seg_off[s, b]) - gbase) // 128
                            for k in range(nb // 128):
                                if jj >= lim and jj < c_sb - 1:
                                    jj += 1
                                    continue
                                nc.tensor.matmul(
                                    out=pa[:], lhsT=gat[:, lc0 + k, :],
                                    rhs=mask_t[:, jj, :],
                                    start=(jj == 0), stop=(jj == c_sb - 1))
                                jj += 1
                        aggs = vp.tile([P, SB_NODES], bf16, tag="aggs")
                        nc.vector.tensor_tensor(
                            out=aggs[:], in0=pa[:],
                            in1=recip_t[:, s * SB_NODES:(s + 1) * SB_NODES],
                            op=mybir.AluOpType.mult)

                        hpT = psM.tile([P, SB_NODES], f32, tag="hpT")
                        nc.tensor.matmul(out=hpT[:], lhsT=wl[:], rhs=aggs[:],
                                         start=True, stop=False)
                        nc.tensor.matmul(
                            out=hpT[:], lhsT=wr[:],
                            rhs=selfT[:, s * SB_NODES:(s + 1) * SB_NODES],
                            start=False, stop=True)

                        if rnd == 0:
                            # relu into the SBUF-resident h1selfT slice
                            cols = slice(s * SB_NODES, (s + 1) * SB_NODES)
                            nc.scalar.activation(
                                h1selfT_t[:, cols],
                                hpT[:], mybir.ActivationFunctionType.Relu,
                                bias=brow[:])
                            tp = psD.tile([P, P], bf16, tag="tp")
                            nc.tensor.transpose(
                                out=tp[:], in_=h1selfT_t[:, cols],
                                identity=consts["ident"][:])
                            h1row = vp.tile([P, P], bf16, tag="h1row")
                            nc.vector.tensor_copy(out=h1row[:], in_=tp[:])
                            nc.sync.dma_start(
                                h1tab_in[s * SB_NODES:(s + 1) * SB_NODES, :],
                                h1row[:])
                        else:
                            h2T = vp.tile([P, SB_NODES], bf16, tag="h2T")
                            nc.scalar.activation(
                                h2T[:], hpT[:],
                                mybir.ActivationFunctionType.Copy)
                            d3 = []
                            for half in range(2):
                                dp = psD.tile([P, SB_NODES], f32, tag="dp")
                                nc.tensor.matmul(
                                    out=dp[:],
                                    lhsT=consts["W3T"][:, half * P:(half + 1) * P],
                                    rhs=h2T[:], start=True, stop=True)
                                ds = vp.tile([P, SB_NODES], bf16,
                                             tag=f"d3{half}")
                                nc.scalar.activation(
                                    ds[:], dp[:],
                                    mybir.ActivationFunctionType.Relu,
                                    bias=consts["b3c"][:, half:half + 1])
                                d3.append(ds)
                            op = psB.tile([1, SB_NODES], f32, tag="op")
                            nc.tensor.matmul(out=op[:],
                                             lhsT=consts["W4c"][:, 0:1],
                                             rhs=d3[0][:], start=True,
                                             stop=False)
                            nc.tensor.matmul(out=op[:],
                                             lhsT=consts["W4c"][:, 1:2],
                                             rhs=d3[1][:], start=False,
                                             stop=True)
                            orow = vp.tile([1, SB_NODES], f32, tag="orow")
                            nc.scalar.activation(
                                orow[:], op[:],
                                mybir.ActivationFunctionType.Copy,
                                bias=meta["b4"])
                            nc.sync.dma_start(out_shard[s:s + 1, :], orow[:])

            emit_round(0, x_tab, consts["W1_lT"], consts["W1_rT"],
                       consts["b1c"])

            # AllGather in 4 row-chunks (quarters) so round-2 gathers can
            # start bank-by-bank as chunks land.
            for q in range(0 if "ag" in ABLATE else 4):
                nc.gpsimd.collective_compute(
                    "AllGather", mybir.AluOpType.bypass,
                    replica_groups=[list(range(N_CORES))],
                    ins=[h1tab_in[q * QCAP:(q + 1) * QCAP, :]],
                    outs=[h1tab[q * N_CORES * QCAP:(q + 1) * N_CORES * QCAP, :]],
                )

            emit_round(1, h1tab, consts["W2_lT"], consts["W2_rT"],
                       consts["b1c"])

    nc.compile()
    return nc


def make_in_maps(meta):
    sm = meta["sm"]
    common = {"x_tab": meta["x_pad"],
              **{k: meta[k] for k in WNAMES}}
    maps = []
    for c in range(N_CORES):
        maps.append({
            **common,
            "x_selfT": meta["x_selfT"][c],
            "recip_bc": meta["recip_bc"][c],
            "idx": sm["idx"][c],
            "mask": sm["mask"][c],
        })
    return maps


_CACHE = {}


def _get_compiled(inputs, n_cores=8):
    assert n_cores == N_CORES
    meta = prep(inputs)
    key = (meta["sm"]["total_slots"],)
    if key not in _CACHE:
        _CACHE[key] = build(meta)
    return _CACHE[key], meta


def kernel(**inputs) -> np.ndarray:
    nc, meta = _get_compiled(inputs)
    in_maps = make_in_maps(meta)
    res = run_bass_kernel_spmd(nc, in_maps, core_ids=list(range(N_CORES)))
    out = np.empty(N_CORES * SHARD, dtype=np.float32)
    rl = _pad_local(np.arange(SHARD))
    for c in range(N_CORES):
        full = res.results[c]["out_shard"].reshape(-1)
        out[c * SHARD:(c + 1) * SHARD] = full[rl]
    return out


# revision 8
# speedup vs baseline: 1.0424x; 1.0424x over previous
"""GraphSAGE (2x SAGEConv mean-aggr + MLP decoder) on 8 Trainium2 NeuronCores.

v3 design (sim-trace driven; v2 was balanced PE/Pool/SP at ~80-93% each):
- dst-node sharding, 12500/core padded to 12800 (4 quarters x 3200), unified
  node numbering shared by both rounds (quarter-major AllGather layout).
- SB_NODES=128 (was 256): halves one-hot mask bytes AND the per-chunk PE
  matmul cost (rhs columns).
- Gathers merged into groups of GS=10 superbatches: 4 bank-calls per group
  (80 total vs 400) to amortize the ~1us fixed SWDGE cost per dma_gather.
  Gather slot layout is group-major/bank-major/sb-major; the fp8 one-hot
  masks are stored sb-major so each sb's mask is ONE contiguous DMA; the
  matmul loop pairs gat chunks with mask chunks via host-computed offsets.
- Mask DMAs alternate between the SP and ACT HWDGE rings (two physical
  rings; v2 serialized everything on SP at 93% busy).
- recip kept on 1 partition and partition-broadcast in the mean multiply
  (saves 25KB/partition of SBUF, enabling GS=10).
- Aggregation: per chunk one matmul lhsT=gat[slot,feat] rhs=mask[slot,128]
  accumulated into fp32 PSUM aggT[feat,dst]; mean via DVE multiply with
  broadcast 1/deg; linears in T-orientation; round 1 relu -> SBUF-resident
  h1selfT -> PE transpose -> h1tab_in; 4-chunk AllGather; decoder fused.
"""

import os

import numpy as np
import ml_dtypes

import concourse.bacc as bacc
import concourse.bass as bass
import concourse.mybir as mybir
import concourse.tile as tile
from concourse.bass_utils import run_bass_kernel_spmd
from concourse.library_config import mlp as mlp_lib

BF16 = ml_dtypes.bfloat16
FP8 = ml_dtypes.float8_e4m3fn

ABLATE = set(os.environ.get("K2_ABLATE", "").split(","))

N_CORES = 8
D = 128
P = 128
SB_NODES = 128
BANK = 25600

SHARD = 12500
QCAP = 3200           # quarter capacity (multiple of 128)
SHARD_PAD = 4 * QCAP  # 12800
N_PAD = N_CORES * SHARD_PAD  # 102400
N_SB = SHARD_PAD // SB_NODES  # 100
N_BANKS = (N_PAD + BANK - 1) // BANK  # 4
GS = 10               # superbatches per gather group
N_GROUPS = N_SB // GS  # 10


def _pad_local(r):
    """local node index [0,12500) -> quarter-padded [0,12800)."""
    q = r // 3125
    return q * QCAP + (r - q * 3125)


def _unified_row(v):
    """global node id -> row in the unified padded table.

    Quarter-major: row = q*8*QCAP + core*QCAP + r_within_quarter, matching
    the layout the 4-chunk AllGather produces (chunk q = concat over cores
    of their quarter q), so AG chunk q fills exactly gather bank q."""
    c = v // SHARD
    r = v - c * SHARD
    q = r // 3125
    rq = r - q * 3125
    return q * (N_CORES * QCAP) + c * QCAP + rq


def _slot_meta(src_row, dst_pad, core_of_edge):
    """Group each core's edges by (sb, bank, dst); pad per-(sb,bank)
    segments to a common (max-over-cores, 128-aligned) budget.

    Gather slot order: group-major, bank-major within group, sb-major
    within bank (so each (group, bank) is one contiguous dma_gather).
    Mask slot order: sb-major (so each sb's mask is one contiguous DMA).
    Returns per-core idx (int16 wrapped) + per-core fp8 one-hot masks
    [128, n_chunks, SB_NODES] + shared budgets/offsets."""
    sb = dst_pad // SB_NODES
    bank = src_row // BANK

    counts = np.zeros((N_CORES, N_SB, N_BANKS), dtype=np.int64)
    np.add.at(counts, (core_of_edge, sb, bank), 1)
    budgets = counts.max(axis=0)
    budgets = ((budgets + 127) // 128) * 128

    seg_off = np.zeros((N_SB, N_BANKS), dtype=np.int64)
    group_base = np.zeros(N_GROUPS + 1, dtype=np.int64)
    pos = 0
    for g in range(N_GROUPS):
        group_base[g] = pos
        for b in range(N_BANKS):
            for s in range(g * GS, (g + 1) * GS):
                seg_off[s, b] = pos
                pos += budgets[s, b]
    group_base[N_GROUPS] = pos
    total_slots = int(pos)
    n_chunks = total_slots // 128

    mask_seg_off = np.zeros((N_SB, N_BANKS), dtype=np.int64)
    mpos = 0
    for s in range(N_SB):
        for b in range(N_BANKS):
            mask_seg_off[s, b] = mpos
            mpos += budgets[s, b]
    assert mpos == total_slots

    idx_cores, mask_cores = [], []
    for c in range(N_CORES):
        m = core_of_edge == c
        s_c, dp_c, sb_c, bk_c = (src_row[m], dst_pad[m], sb[m], bank[m])
        order = np.lexsort((dp_c, bk_c, sb_c))
        s_c, dp_c, sb_c, bk_c = (a[order] for a in (s_c, dp_c, sb_c, bk_c))

        idx_full = np.zeros(total_slots, dtype=np.int16)
        dstw_full = np.full(total_slots, -1, dtype=np.int64)
        cnt_c = np.zeros((N_SB, N_BANKS), dtype=np.int64)
        np.add.at(cnt_c, (sb_c, bk_c), 1)
        # edges are sorted (sb, bank): per-segment start in that order
        seg_start = np.zeros((N_SB, N_BANKS), dtype=np.int64)
        seg_start.reshape(-1)[1:] = np.cumsum(cnt_c.reshape(-1))[:-1]
        pos_in_seg = np.arange(len(s_c)) - seg_start[sb_c, bk_c]
        gslot = seg_off[sb_c, bk_c] + pos_in_seg
        mslot = mask_seg_off[sb_c, bk_c] + pos_in_seg
        idx_full[gslot] = (s_c - bk_c * BANK).astype(np.int16)
        dstw_full[mslot] = dp_c - sb_c * SB_NODES

        # idx wrap: slot i -> [i%16, i//16], replicated to 128 partitions
        w = idx_full.reshape(total_slots // 16, 16).T
        idx_cores.append(np.tile(w, (8, 1)).copy())

        # fp8 one-hot mask (sb-major): slot i -> partition i%128, chunk i//128
        dw = dstw_full.reshape(n_chunks, 128).T          # [128, chunks]
        mask = np.zeros((P, n_chunks, SB_NODES), dtype=FP8)
        valid = dw >= 0
        np.put_along_axis(mask, dw.clip(0)[:, :, None],
                          valid[:, :, None].astype(FP8), axis=2)
        mask_cores.append(mask)

    return {
        "budgets": budgets, "seg_off": seg_off, "mask_seg_off": mask_seg_off,
        "group_base": group_base, "total_slots": total_slots,
        "n_chunks": n_chunks, "idx": idx_cores, "mask": mask_cores,
    }


def prep(inputs):
    x = np.asarray(inputs["x"], dtype=np.float32)
    ei = np.asarray(inputs["edge_index"])
    n = x.shape[0]
    assert n == N_CORES * SHARD

    src = ei[0].astype(np.int64)
    dst = ei[1].astype(np.int64)
    src_row = _unified_row(src)
    core_of_edge = dst // SHARD
    dst_local = dst - core_of_edge * SHARD
    dst_pad = _pad_local(dst_local)

    sm = _slot_meta(src_row, dst_pad, core_of_edge)

    deg = np.bincount(dst, minlength=n).astype(np.float32)
    recip = (1.0 / np.maximum(deg, 1.0)).astype(np.float32)
    recip_pad = np.zeros((N_CORES, 1, SHARD_PAD), dtype=np.float32)
    x_selfT = np.zeros((N_CORES, D, SHARD_PAD), dtype=np.float32)
    for c in range(N_CORES):
        rl = _pad_local(np.arange(SHARD))
        recip_pad[c, 0, rl] = recip[c * SHARD:(c + 1) * SHARD]
        x_selfT[c, :, rl] = x[c * SHARD:(c + 1) * SHARD, :]

    x_pad = np.zeros((N_PAD, D), dtype=np.float32)
    x_pad[_unified_row(np.arange(n))] = x

    meta = {
        "sm": sm,
        "x_pad": x_pad.astype(BF16),
        "x_selfT": x_selfT.astype(BF16),
        "recip": recip_pad.astype(BF16),
        "W1_lT": np.asarray(inputs["W1_l"], np.float32).T.astype(BF16).copy(),
        "W1_rT": np.asarray(inputs["W1_r"], np.float32).T.astype(BF16).copy(),
        "W2_lT": np.asarray(inputs["W2_l"], np.float32).T.astype(BF16).copy(),
        "W2_rT": np.asarray(inputs["W2_r"], np.float32).T.astype(BF16).copy(),
        "W3T": np.asarray(inputs["W3"], np.float32).T.astype(BF16).copy(),
        "W4c": np.asarray(inputs["W4"], np.float32).reshape(2, 128).T
            .astype(BF16).copy(),
        "b1c": np.asarray(inputs["b1"], np.float32).reshape(-1, 1)
            .astype(BF16).copy(),
        # conv2 has no relu, so b2 folds exactly into the decoder bias:
        # relu(W3 @ (h2 + b2) + b3) = relu(W3 @ h2 + (b3 + W3 @ b2))
        "b3c": (np.asarray(inputs["b3"], np.float32)
                + np.asarray(inputs["W3"], np.float32)
                @ np.asarray(inputs["b2"], np.float32))
            .reshape(2, 128).T.astype(BF16).copy(),
        "b4": float(np.asarray(inputs["b4"]).reshape(-1)[0]),
        "ident": np.eye(P, dtype=BF16),
    }
    meta["recip_bc"] = np.broadcast_to(
        meta["recip"], (N_CORES, P, SHARD_PAD)).copy()
    return meta


WNAMES = ["W1_lT", "W1_rT", "W2_lT", "W2_rT", "W3T", "W4c",
          "b1c", "b3c", "ident"]


def build(meta):
    sm = meta["sm"]
    nc = bacc.Bacc("TRN2", target_bir_lowering=False, debug=False,
                   num_devices=N_CORES, num_swdge_queues=4)
    f32, bf16, fp8 = mybir.dt.float32, mybir.dt.bfloat16, mybir.dt.float8e4
    i16 = mybir.dt.int16

    x_tab = nc.dram_tensor("x_tab", [N_PAD, D], bf16, kind="ExternalInput")
    x_selfT_d = nc.dram_tensor("x_selfT", [D, SHARD_PAD], bf16,
                               kind="ExternalInput")
    recip_d = nc.dram_tensor("recip_bc", [P, SHARD_PAD], bf16,
                              kind="ExternalInput")
    idx_d = nc.dram_tensor("idx", list(sm["idx"][0].shape), i16,
                           kind="ExternalInput")
    mask_d = nc.dram_tensor("mask", [P, sm["n_chunks"], SB_NODES], fp8,
                            kind="ExternalInput")
    wt = {}
    for name in WNAMES:
        arr = meta[name]
        wt[name] = nc.dram_tensor(name, list(arr.shape), bf16,
                                  kind="ExternalInput")

    h1tab_in = nc.dram_tensor("h1tab_in", [SHARD_PAD, D], bf16)
    h1tab = nc.dram_tensor("h1tab", [N_PAD, D], bf16,
                           addr_space="Shared")
    out_shard = nc.dram_tensor("out_shard", [N_SB, SB_NODES], f32,
                               kind="ExternalOutput")

    budgets, seg_off = sm["budgets"], sm["seg_off"]
    mask_seg_off, group_base = sm["mask_seg_off"], sm["group_base"]

    with tile.TileContext(nc) as tc:
        with (
            tc.tile_pool(name="wp", bufs=1) as wp,
            tc.tile_pool(name="gp", bufs=2) as gp,
            tc.tile_pool(name="sp", bufs=4) as sp,
            tc.tile_pool(name="vp", bufs=3) as vp,
            tc.tile_pool(name="psA", bufs=2, space="PSUM") as psA,
            tc.tile_pool(name="psB", bufs=1, space="PSUM") as psB,
            tc.tile_pool(name="psM", bufs=2, space="PSUM") as psM,
            tc.tile_pool(name="psD", bufs=1, space="PSUM") as psD,
        ):
            with tc.tile_critical():
                nc.gpsimd.load_library(mlp_lib)

            consts = {}
            for name in WNAMES:
                t = wp.tile(list(meta[name].shape), bf16, tag=name)
                nc.sync.dma_start(t[:], wt[name][:])
                consts[name] = t
            recip_t = wp.tile([P, SHARD_PAD], bf16, tag="recip")
            nc.sync.dma_start(recip_t[:], recip_d[:])
            x_selfT_t = wp.tile([D, SHARD_PAD], bf16, tag="xselfT")
            nc.sync.dma_start(x_selfT_t[:], x_selfT_d[:])
            h1selfT_t = wp.tile([D, SHARD_PAD], bf16, tag="h1selfT")

            def emit_round(rnd, table, wl, wr, brow):
                selfT = x_selfT_t if rnd == 0 else h1selfT_t
                for g in range(N_GROUPS):
                    gbase = int(group_base[g])
                    gslots = int(group_base[g + 1]) - gbase

                    idx_t = gp.tile([P, gslots // 16], i16, tag="idx")
                    nc.sync.dma_start(
                        idx_t[:],
                        idx_d[:, gbase // 16: (gbase + gslots) // 16])
                    gat = gp.tile([P, gslots // 128, D], bf16, tag="gat")
                    for b in range(N_BANKS):
                        nb = int(budgets[g * GS:(g + 1) * GS, b].sum())
                        if nb == 0 or "gather" in ABLATE:
                            continue
                        off = int(seg_off[g * GS, b]) - gbase
                        lo = b * BANK
                        hi = min(N_PAD, (b + 1) * BANK)
                        nc.gpsimd.dma_gather(
                            gat[:, off // 128: (off + nb) // 128, :],
                            table[lo:hi, :],
                            idx_t[:, off // 16: (off + nb) // 16],
                            num_idxs=nb, num_idxs_reg=nb, elem_size=D,
                            single_packet=False, queue_num=b,
                        )

                    for s in range(g * GS, (g + 1) * GS):
                        c_sb = int(budgets[s].sum()) // 128
                        mc0 = int(mask_seg_off[s, 0]) // 128
                        mask_t = sp.tile([P, c_sb, SB_NODES], fp8, tag="mask")
                        nc.sync.dma_start(mask_t[:], mask_d[:, mc0:mc0 + c_sb, :])

                        pa = psA.tile([P, SB_NODES], f32, tag="pa")
                        jj = 0
                        lim = 1 if "agg" in ABLATE else c_sb
                        for b in range(N_BANKS):
                            nb = int(budgets[s, b])
                            lc0 = (int(seg_off[s, b]) - gbase) // 128
                            for k in range(nb // 128):
                                if jj >= lim and jj < c_sb - 1:
                                    jj += 1
                                    continue
                                nc.tensor.matmul(
                                    out=pa[:], lhsT=gat[:, lc0 + k, :],
                                    rhs=mask_t[:, jj, :],
                                    start=(jj == 0), stop=(jj == c_sb - 1))
                                jj += 1
                        aggs = vp.tile([P, SB_NODES], bf16, tag="aggs")
                        nc.vector.tensor_tensor(
                            out=aggs[:], in0=pa[:],
                            in1=recip_t[:, s * SB_NODES:(s + 1) * SB_NODES],
                            op=mybir.AluOpType.mult)

                        hpT = psM.tile([P, SB_NODES], f32, tag="hpT")
                        nc.tensor.matmul(out=hpT[:], lhsT=wl[:], rhs=aggs[:],
                                         start=True, stop=False)
                        nc.tensor.matmul(
                            out=hpT[:], lhsT=wr[:],
                            rhs=selfT[:, s * SB_NODES:(s + 1) * SB_NODES],
                            start=False, stop=True)

                        if rnd == 0:
                            # relu into the SBUF-resident h1selfT slice
                            cols = slice(s * SB_NODES, (s + 1) * SB_NODES)
                            nc.scalar.activation(
                                h1selfT_t[:, cols],
                                hpT[:], mybir.ActivationFunctionType.Relu,
                                bias=brow[:])
                            tp = psD.tile([P, P], bf16, tag="tp")
                            nc.tensor.transpose(
                                out=tp[:], in_=h1selfT_t[:, cols],
                                identity=consts["ident"][:])
                            h1row = vp.tile([P, P], bf16, tag="h1row")
                            nc.vector.tensor_copy(out=h1row[:], in_=tp[:])
                            nc.sync.dma_start(
                                h1tab_in[s * SB_NODES:(s + 1) * SB_NODES, :],
                                h1row[:])
                        else:
                            h2T = vp.tile([P, SB_NODES], bf16, tag="h2T")
                            nc.scalar.activation(
                                h2T[:], hpT[:],
                                mybir.ActivationFunctionType.Copy)
                            d3 = []
                            for half in range(2):
                                dp = psD.tile([P, SB_NODES], f32, tag="dp")
                                nc.tensor.matmul(
                                    out=dp[:],
                                    lhsT=consts["W3T"][:, half * P:(half + 1) * P],
                                    rhs=h2T[:], start=True, stop=True)
                                ds = vp.tile([P, SB_NODES], bf16,
                                             tag=f"d3{half}")
                                nc.scalar.activation(
                                    ds[:], dp[:],
                                    mybir.ActivationFunctionType.Relu,
                                    bias=consts["b3c"][:, half:half + 1])
                                d3.append(ds)
                            op = psB.tile([1, SB_NODES], f32, tag="op")
                            nc.tensor.matmul(out=op[:],
                                             lhsT=consts["W4c"][:, 0:1],
                                             rhs=d3[0][:], start=True,
                                             stop=False)
                            nc.tensor.matmul(out=op[:],
                                             lhsT=consts["W4c"][:, 1:2],
                                             rhs=d3[1][:], start=False,
                                             stop=True)
                            orow = vp.tile([1, SB_NODES], f32, tag="orow")
                            nc.scalar.activation(
                                orow[:], op[:],
                                mybir.ActivationFunctionType.Copy,
                                bias=meta["b4"])
                            nc.sync.dma_start(out_shard[s:s + 1, :], orow[:])

            emit_round(0, x_tab, consts["W1_lT"], consts["W1_rT"],
                       consts["b1c"])

            # AllGather in 4 row-chunks (quarters) so round-2 gathers can
            # start bank-by-bank as chunks land.
            for q in range(0 if "ag" in ABLATE else 4):
                nc.gpsimd.collective_compute(
                    "AllGather", mybir.AluOpType.bypass,
                    replica_groups=[list(range(N_CORES))],
                    ins=[h1tab_in[q * QCAP:(q + 1) * QCAP, :]],
                    outs=[h1tab[q * N_CORES * QCAP:(q + 1) * N_CORES * QCAP, :]],
                )

            emit_round(1, h1tab, consts["W2_lT"], consts["W2_rT"],
                       consts["b1c"])

    nc.compile()
    return nc


def make_in_maps(meta):
    sm = meta["sm"]
    common = {"x_tab": meta["x_pad"],
              **{k: meta[k] for k in WNAMES}}
    maps = []
    for c in range(N_CORES):
        maps.append({
            **common,
            "x_selfT": meta["x_selfT"][c],
            "recip_bc": meta["recip_bc"][c],
            "idx": sm["idx"][c],
            "mask": sm["mask"][c],
        })
    return maps


_CACHE = {}


def _get_compiled(inputs, n_cores=8):
    assert n_cores == N_CORES
    meta = prep(inputs)
    key = (meta["sm"]["total_slots"],)
    if key not in _CACHE:
        _CACHE[key] = build(meta)
    return _CACHE[key], meta


def kernel(**inputs) -> np.ndarray:
    nc, meta = _get_compiled(inputs)
    in_maps = make_in_maps(meta)
    res = run_bass_kernel_spmd(nc, in_maps, core_ids=list(range(N_CORES)))
    out = np.empty(N_CORES * SHARD, dtype=np.float32)
    rl = _pad_local(np.arange(SHARD))
    for c in range(N_CORES):
        full = res.results[c]["out_shard"].reshape(-1)
        out[c * SHARD:(c + 1) * SHARD] = full[rl]
    return out
